# revision 9
# baseline (speedup 1.0000x reference)
"""Trainium2 Bass kernel: custom inverse STFT (degenerate per-bin rotation +
Hann window + overlap-add + window correction).

Math (matching the reference):
    F[i,k]  = S_real[i,k]*A[k] + S_imag[i,k]*B[k]
      A[k]  = w[k]*(cos(th)-sin(th))/n,  B[k] = -w[k]*(cos(th)+sin(th))/n
    out[t]  = sum_i F[i, t-256*i] / max(corr[t], 1e-8)

Implementation (fp16 inputs, bf16 products, f32 accumulation):
  - Inputs are cast to fp16 on the host (halves HBM traffic; the 2e-2 rel-err
    budget dwarfs 16-bit rounding).  Coefficients are scaled by 2^8 so
    products stay in fp16's normal range; the host divides the scale out.
  - Sharding: 8192 frames -> 8 cores x 1024 frames (+3 left-halo frames).
  - Per core: 9 slices of 128 consecutive frames starting at multiples of
    125 (slices overlap by 3 frames), frame = partition.  Each slice yields
    125 output blocks whose 4 overlap-add contributions all live on
    partitions of the SAME slice, so the whole overlap-add (including the
    t1+t2 sum) runs on the TensorEngine as shifted-identity matmuls
    accumulating exactly in f32 PSUM.  Products are written as bf16 (PE
    matmul is full-rate for bf16, half-rate for fp16); the DVE does the two
    products per slice in 2x packed mode; ACT stages PSUM->SBUF fp16.
  - Input DMAs are per-slice contiguous 256KB transfers; the output is
    stored in device-natural [125, 9*256] order (contiguous per partition)
    and reordered on the host.
  - The 6 edge blocks (corr < 2 there: the window-correction division
    amplifies fp16 underflow by up to ~1e5) and the 768-sample global tail
    are recomputed exactly in f32 on the host.
"""

import numpy as np
import ml_dtypes

import concourse.bass as bass
import concourse.bacc as bacc
import concourse.mybir as mybir
import concourse.tile as tile
from concourse.bass_utils import run_bass_kernel_spmd

F16 = mybir.dt.float16
BF16 = mybir.dt.bfloat16
F32 = mybir.dt.float32
ALU = mybir.AluOpType

P = 128            # SBUF partitions
FL = 1024          # frame length (== fft length)
FS = 256           # frame step
NF = 8192          # total frames
NCORES = 8
FPC = NF // NCORES          # frames owned per core (1024)
ROWS = FPC + 3              # input rows per core (3 left-halo frames)
SL = 125                    # slice stride in frames (overlap of 3)
NS = 9                      # slices (8 full x 125 blocks + 1 partial x 24)
OUT_LEN = FS * (NF - 1) + FL
SCALE = np.float32(256.0)   # exact power-of-2 coefficient pre-scale


def _window32():
    # bit-matches the reference's f32 window computation
    k = np.arange(FL, dtype=np.float32)
    th = np.float32(2.0 * np.pi) * k / np.float32(FL)
    return (np.float32(0.5) - np.float32(0.5) * np.cos(th)).astype(np.float32)


def _coeffs32():
    k = np.arange(FL, dtype=np.float64)
    th = 2.0 * np.pi * k / FL
    w = _window32().astype(np.float64)
    a = (w * (np.cos(th) - np.sin(th)) / FL).astype(np.float32)
    b = (-w * (np.cos(th) + np.sin(th)) / FL).astype(np.float32)
    return a, b


def _window_correction():
    w = _window32()
    corr = np.zeros(OUT_LEN, dtype=np.float32)
    for j in range(4):
        view = corr[j * FS:j * FS + NF * FS].reshape(NF, FS)
        view += w[j * FS:(j + 1) * FS][None, :]
    return corr


def build_nc():
    nc = bacc.Bacc(trn_type="TRN2", target_bir_lowering=False, debug=False)
    sr_d = nc.dram_tensor("s_real", [ROWS, FL], F16, kind="ExternalInput").ap()
    si_d = nc.dram_tensor("s_imag", [ROWS, FL], F16, kind="ExternalInput").ap()
    cf_d = nc.dram_tensor("coefs", [2 * FL + P], F16, kind="ExternalInput").ap()
    id_d = nc.dram_tensor("ident", [P, P], BF16, kind="ExternalInput").ap()
    out_d = nc.dram_tensor("out_seg", [SL * NS * FS], F16, kind="ExternalOutput").ap()

    with tile.TileContext(nc) as tc:
        with (
            tc.tile_pool(name="const", bufs=1) as cpool,
            tc.tile_pool(name="main", bufs=1) as mpool,
            tc.tile_pool(name="psum", bufs=1, space="PSUM") as ppool,
        ):
            Crow = cpool.tile([1, 2 * FL + P], F16, tag="Crow")  # A*2^8|B*2^8|ones
            Id = cpool.tile([P, P], BF16, tag="Id")
            At = cpool.tile([P, FL], F16, tag="At")
            Bt = cpool.tile([P, FL], F16, tag="Bt")
            Srt = mpool.tile([P, NS * FL], F16, tag="Sr")
            Sit = mpool.tile([P, NS * FL], F16, tag="Si")
            T1 = mpool.tile([P, NS * FL], BF16, tag="T1")
            T2 = mpool.tile([P, NS * FL], BF16, tag="T2")
            Ot = mpool.tile([P, NS * FS], F16, tag="Ot")
            ABp = ppool.tile([P, FL], F32, tag="ABp")
            Op = ppool.tile([P, NS * FS], F32, tag="Op")

            # constants first on the sync queue so the coefficient broadcast
            # completes while the input streams
            nc.sync.dma_start(out=Crow[:, :], in_=cf_d[None, :])
            nc.sync.dma_start(out=Id[:, :], in_=id_d[:, :])

            # slice 8 is partial (27 of 128 rows DMA'd); zero it so its
            # products/outputs are clean zeros instead of SBUF garbage
            nc.gpsimd.memset(Srt[:, 8 * FL:9 * FL], 0.0)
            nc.gpsimd.memset(Sit[:, 8 * FL:9 * FL], 0.0)

            # input stream: per-slice DMAs; each reads a CONTIGUOUS 256KB
            # DRAM range (rows s*125 .. s*125+127).  Sr on sync, Si on ACT.
            for s in range(NS):
                r0 = s * SL
                nrow = P if s < 8 else ROWS - 8 * SL
                dst = Srt[0:nrow, s * FL:(s + 1) * FL]
                nc.sync.dma_start(out=dst, in_=sr_d[r0:r0 + nrow, :])
                dst = Sit[0:nrow, s * FL:(s + 1) * FL]
                nc.scalar.dma_start(out=dst, in_=si_d[r0:r0 + nrow, :])

            # broadcast A,B to all partitions via K=1 matmul (ones @ row)
            ones = Crow[0:1, 2 * FL:2 * FL + P]
            for h in range(2):
                nc.tensor.matmul(ABp[:, h * 512:(h + 1) * 512], ones,
                                 Crow[0:1, h * 512:(h + 1) * 512],
                                 start=True, stop=True)
            nc.vector.tensor_copy(out=At[:, :], in_=ABp[:, :])
            for h in range(2):
                nc.tensor.matmul(ABp[:, h * 512:(h + 1) * 512], ones,
                                 Crow[0:1, FL + h * 512:FL + (h + 1) * 512],
                                 start=True, stop=True)
            nc.vector.tensor_copy(out=Bt[:, :], in_=ABp[:, :])

            # products on the DVE (fp16 x fp16 -> bf16, 2x packed mode)
            for s in range(NS):
                sl = slice(s * FL, (s + 1) * FL)
                nc.vector.tensor_tensor(out=T1[:, sl], in0=Srt[:, sl],
                                        in1=At[:, :], op=ALU.mult)
                nc.vector.tensor_tensor(out=T2[:, sl], in0=Sit[:, sl],
                                        in1=Bt[:, :], op=ALU.mult)

            # overlap-add on the PE: slice s, output block m (0..124), chunk c
            # reads frame q = m+3-c of the same slice -> lhsT = Id[:, 3-c:128-c]
            T1v = T1[:, :].rearrange("p (s k) -> p s k", s=NS)
            T2v = T2[:, :].rearrange("p (s k) -> p s k", s=NS)
            pairs = [(0, 2), (2, 2), (4, 2), (6, 2), (8, 1)]
            for s0, nsl in pairs:
                osl = slice(s0 * FS, (s0 + nsl) * FS)
                for c in range(4):
                    ksl = slice(c * FS, (c + 1) * FS)
                    w = Id[:, 3 - c:3 - c + SL]
                    nc.tensor.matmul(Op[0:SL, osl], w, T1v[:, s0:s0 + nsl, ksl],
                                     start=(c == 0), stop=False)
                    nc.tensor.matmul(Op[0:SL, osl], w, T2v[:, s0:s0 + nsl, ksl],
                                     start=False, stop=(c == 3))
                # PSUM -> SBUF fp16 staging on the ACT engine
                nc.scalar.copy(out=Ot[0:SL, osl], in_=Op[0:SL, osl])

            # store in device-natural order (contiguous per partition);
            # the host de-interleaves blocks (block s*125+m lives at
            # seg[m, s*256:(s+1)*256])
            dst = out_d.rearrange("(p x) -> p x", p=SL)
            nc.scalar.dma_start(out=dst, in_=Ot[0:SL, :])
    nc.compile()
    return nc


_cache = {}


def _get_nc():
    if "nc" not in _cache:
        _cache["nc"] = build_nc()
    return _cache["nc"]


def make_in_maps(S_real, S_imag):
    a32, b32 = _coeffs32()
    coefs = np.zeros(2 * FL + P, dtype=np.float16)
    coefs[0:FL] = (a32 * SCALE).astype(np.float16)
    coefs[FL:2 * FL] = (b32 * SCALE).astype(np.float16)
    coefs[2 * FL:] = np.float16(1.0)
    ident = np.eye(P, dtype=ml_dtypes.bfloat16)

    pad = np.zeros((3, FL), dtype=np.float16)
    sr16 = np.concatenate([pad, S_real.astype(np.float16)], axis=0)
    si16 = np.concatenate([pad, S_imag.astype(np.float16)], axis=0)

    in_maps = []
    for m in range(NCORES):
        r0 = m * FPC
        in_maps.append({
            "s_real": np.ascontiguousarray(sr16[r0:r0 + ROWS]),
            "s_imag": np.ascontiguousarray(si16[r0:r0 + ROWS]),
            "coefs": coefs,
            "ident": ident,
        })
    return in_maps


def assemble_output(S_real, S_imag, segs):
    a32, b32 = _coeffs32()
    full = np.empty(OUT_LEN, dtype=np.float32)
    inv_scale = np.float32(1.0) / SCALE
    for m in range(NCORES):
        # seg[m, s*256+r] -> block s*125+m; de-interleave then trim to 1024
        v = segs[m].reshape(SL, NS, FS).transpose(1, 0, 2).reshape(-1)[:FPC * FS]
        full[m * FPC * FS:(m + 1) * FPC * FS] = v.astype(np.float32) * inv_scale

    # exact f32 recompute of the 6 edge blocks (corr < 2 there: the final
    # division amplifies fp16 error by up to ~1e5) and the global tail
    Fh = S_real[0:3] * a32[None, :] + S_imag[0:3] * b32[None, :]
    full[0:FS] = Fh[0, 0:FS]
    full[FS:2 * FS] = Fh[0, FS:2 * FS] + Fh[1, 0:FS]
    full[2 * FS:3 * FS] = Fh[0, 2 * FS:3 * FS] + Fh[1, FS:2 * FS] + Fh[2, 0:FS]
    Ft = S_real[NF - 3:] * a32[None, :] + S_imag[NF - 3:] * b32[None, :]
    full[NF * FS:NF * FS + FS] = Ft[0, 3 * FS:] + Ft[1, 2 * FS:3 * FS] + Ft[2, FS:2 * FS]
    full[NF * FS + FS:NF * FS + 2 * FS] = Ft[1, 3 * FS:] + Ft[2, 2 * FS:3 * FS]
    full[NF * FS + 2 * FS:] = Ft[2, 3 * FS:]

    if "corr" not in _cache:
        _cache["corr"] = np.maximum(_window_correction(), np.float32(1e-8))
    return full / _cache["corr"]


def kernel(S_real, S_imag):
    S_real = np.asarray(S_real, dtype=np.float32)
    S_imag = np.asarray(S_imag, dtype=np.float32)
    in_maps = make_in_maps(S_real, S_imag)
    nc = _get_nc()
    res = run_bass_kernel_spmd(nc, in_maps, list(range(NCORES)))
    segs = [res.results[m]["out_seg"] for m in range(NCORES)]
    return assemble_output(S_real, S_imag, segs)


# revision 10
# speedup vs baseline: 1.1467x; 1.1467x over previous
"""Trainium2 Bass kernel: custom inverse STFT (degenerate per-bin rotation +
Hann window + overlap-add + window correction).

Math (matching the reference):
    F[i,k]  = S_real[i,k]*A[k] + S_imag[i,k]*B[k]
      A[k]  = w[k]*(cos(th)-sin(th))/n,  B[k] = -w[k]*(cos(th)+sin(th))/n
    out[t]  = sum_i F[i, t-256*i] / max(corr[t], 1e-8)

Implementation (fp16 inputs, bf16 products, f32 accumulation):
  - Inputs are cast to fp16 on the host (halves HBM traffic; the 2e-2 rel-err
    budget dwarfs 16-bit rounding).  Coefficients are scaled by 2^8 so
    products stay in fp16's normal range; the host divides the scale out.
  - Sharding: 8192 frames -> 8 cores x 1024 frames (+3 left-halo frames).
  - Per core: 9 slices of 128 consecutive frames starting at multiples of
    125 (slices overlap by 3 frames), frame = partition.  Each slice yields
    125 output blocks whose 4 overlap-add contributions all live on
    partitions of the SAME slice, so the whole overlap-add (including the
    t1+t2 sum) runs on the TensorEngine as shifted-identity matmuls
    accumulating exactly in f32 PSUM.  Products are written as bf16 (PE
    matmul is full-rate for bf16, half-rate for fp16).
  - The host pre-packs the input as [128, 9, 2048] fp16 (slice rows on
    partitions, Sr|Si interleaved, overlap rows duplicated, padding zeroed)
    so every DMA moves 4KB-contiguous per-partition segments, and the DVE
    computes both products of a slice in ONE 2x-packed op against the
    broadcast [A|B] row.
  - Output is stored in device-natural [128, 9*256] order (contiguous per
    partition) and de-interleaved on the host.
  - The 6 edge blocks (corr < 2 there: the window-correction division
    amplifies fp16 underflow by up to ~1e5) and the 768-sample global tail
    are recomputed exactly in f32 on the host.
"""

import numpy as np
import ml_dtypes

import concourse.bass as bass
import concourse.bacc as bacc
import concourse.mybir as mybir
import concourse.tile as tile
from concourse.bass_utils import run_bass_kernel_spmd

F16 = mybir.dt.float16
BF16 = mybir.dt.bfloat16
F32 = mybir.dt.float32
ALU = mybir.AluOpType

P = 128            # SBUF partitions
FL = 1024          # frame length (== fft length)
FS = 256           # frame step
NF = 8192          # total frames
NCORES = 8
FPC = NF // NCORES          # frames owned per core (1024)
ROWS = FPC + 3              # input rows per core (3 left-halo frames)
SL = 125                    # slice stride in frames (overlap of 3)
NS = 9                      # slices (8 full x 125 blocks + 1 partial x 24)
W2 = 2 * FL                 # interleaved Sr|Si row width (2048)
OUT_LEN = FS * (NF - 1) + FL
SCALE = np.float32(256.0)   # exact power-of-2 coefficient pre-scale


def _window32():
    # bit-matches the reference's f32 window computation
    k = np.arange(FL, dtype=np.float32)
    th = np.float32(2.0 * np.pi) * k / np.float32(FL)
    return (np.float32(0.5) - np.float32(0.5) * np.cos(th)).astype(np.float32)


def _coeffs32():
    k = np.arange(FL, dtype=np.float64)
    th = 2.0 * np.pi * k / FL
    w = _window32().astype(np.float64)
    a = (w * (np.cos(th) - np.sin(th)) / FL).astype(np.float32)
    b = (-w * (np.cos(th) + np.sin(th)) / FL).astype(np.float32)
    return a, b


def _window_correction():
    w = _window32()
    corr = np.zeros(OUT_LEN, dtype=np.float32)
    for j in range(4):
        view = corr[j * FS:j * FS + NF * FS].reshape(NF, FS)
        view += w[j * FS:(j + 1) * FS][None, :]
    return corr


def build_nc():
    nc = bacc.Bacc(trn_type="TRN2", target_bir_lowering=False, debug=False)
    x_d = nc.dram_tensor("x", [P * NS * W2], F16, kind="ExternalInput").ap()
    cf_d = nc.dram_tensor("coefs", [W2 + P], F16, kind="ExternalInput").ap()
    id_d = nc.dram_tensor("ident", [P, P], BF16, kind="ExternalInput").ap()
    out_d = nc.dram_tensor("out_seg", [P * NS * FS], F16, kind="ExternalOutput").ap()

    xv = x_d.rearrange("(p s k) -> p s k", p=P, s=NS)   # [128, 9, 2048]

    with tile.TileContext(nc) as tc:
        with (
            tc.tile_pool(name="const", bufs=1) as cpool,
            tc.tile_pool(name="main", bufs=1) as mpool,
            tc.tile_pool(name="psum", bufs=1, space="PSUM") as ppool,
        ):
            Crow = cpool.tile([1, W2 + P], F16, tag="Crow")  # A*2^8|B*2^8|ones
            Id = cpool.tile([P, P], BF16, tag="Id")
            ABt = cpool.tile([P, W2], F16, tag="ABt")
            X = mpool.tile([P, NS * W2], F16, tag="X")
            T = mpool.tile([P, NS * W2], BF16, tag="T")
            Ot = mpool.tile([P, NS * FS], F16, tag="Ot")
            ABp = ppool.tile([P, FL], F32, tag="ABp")
            Op = ppool.tile([P, NS * FS], F32, tag="Op")

            # constants first on the sync queue so the coefficient broadcast
            # completes while the input streams
            nc.sync.dma_start(out=Crow[:, :], in_=cf_d[None, :])
            nc.sync.dma_start(out=Id[:, :], in_=id_d[:, :])

            # input stream: one 512KB DMA per slice (4KB contiguous per
            # partition), alternating between the two HWDGE queues
            for s in range(NS):
                dst = X[:, s * W2:(s + 1) * W2]
                eng = nc.sync if s % 2 == 0 else nc.scalar
                eng.dma_start(out=dst, in_=xv[:, s, :])

            # broadcast [A|B] to all partitions via K=1 matmuls (ones @ row),
            # two rounds through a 2-bank PSUM staging tile
            ones = Crow[0:1, W2:W2 + P]
            for r in range(2):
                for h in range(2):
                    nc.tensor.matmul(ABp[:, h * 512:(h + 1) * 512], ones,
                                     Crow[0:1, r * FL + h * 512:r * FL + (h + 1) * 512],
                                     start=True, stop=True)
                nc.vector.tensor_copy(out=ABt[:, r * FL:(r + 1) * FL], in_=ABp[:, :])

            # products on the DVE: one op per slice computes BOTH
            # t1=Sr*A and t2=Si*B (fp16 x fp16 -> bf16, 2x packed mode)
            for s in range(NS):
                sl = slice(s * W2, (s + 1) * W2)
                nc.vector.tensor_tensor(out=T[:, sl], in0=X[:, sl],
                                        in1=ABt[:, :], op=ALU.mult)

            # overlap-add on the PE: slice s, output block m (0..124), chunk c
            # reads frame q = m+3-c of the same slice -> lhsT = Id[:, 3-c:128-c]
            Tv = T[:, :].rearrange("p (s x) -> p s x", s=NS)
            pairs = [(0, 2), (2, 2), (4, 2), (6, 2), (8, 1)]
            for s0, nsl in pairs:
                osl = slice(s0 * FS, (s0 + nsl) * FS)
                for c in range(4):
                    w = Id[:, 3 - c:3 - c + SL]
                    k1 = slice(c * FS, (c + 1) * FS)
                    k2 = slice(FL + c * FS, FL + (c + 1) * FS)
                    nc.tensor.matmul(Op[0:SL, osl], w, Tv[:, s0:s0 + nsl, k1],
                                     start=(c == 0), stop=False)
                    nc.tensor.matmul(Op[0:SL, osl], w, Tv[:, s0:s0 + nsl, k2],
                                     start=False, stop=(c == 3))
                # PSUM -> SBUF fp16 staging on the ACT engine
                nc.scalar.copy(out=Ot[0:SL, osl], in_=Op[0:SL, osl])

            # store in device-natural order (contiguous per partition, all
            # 128 partitions so the transfer spreads over all SDMA engines);
            # the host de-interleaves (block s*125+m is at seg[m, s, :])
            ov = out_d.rearrange("(p x) -> p x", p=P)
            nc.scalar.dma_start(out=ov[:, 0:4 * FS], in_=Ot[:, 0:4 * FS])
            nc.scalar.dma_start(out=ov[:, 4 * FS:], in_=Ot[:, 4 * FS:])
    nc.compile()
    return nc


_cache = {}


def _get_nc():
    if "nc" not in _cache:
        _cache["nc"] = build_nc()
    return _cache["nc"]


def make_in_maps(S_real, S_imag):
    a32, b32 = _coeffs32()
    coefs = np.zeros(W2 + P, dtype=np.float16)
    coefs[0:FL] = (a32 * SCALE).astype(np.float16)
    coefs[FL:W2] = (b32 * SCALE).astype(np.float16)
    coefs[W2:] = np.float16(1.0)
    ident = np.eye(P, dtype=ml_dtypes.bfloat16)

    # interleaved + padded fp16 input: row r of core m = global frame
    # m*1024 - 3 + r (zeros outside [0, NF))
    sr16 = S_real.astype(np.float16)
    si16 = S_imag.astype(np.float16)
    glob = np.zeros((3 + NF + P, W2), dtype=np.float16)
    glob[3:3 + NF, 0:FL] = sr16
    glob[3:3 + NF, FL:W2] = si16

    in_maps = []
    for m in range(NCORES):
        base = m * FPC
        x = np.empty((P, NS, W2), dtype=np.float16)
        for s in range(NS):
            x[:, s, :] = glob[base + s * SL:base + s * SL + P]
        in_maps.append({
            "x": x.reshape(-1),
            "coefs": coefs,
            "ident": ident,
        })
    return in_maps


def assemble_output(S_real, S_imag, segs):
    a32, b32 = _coeffs32()
    full = np.empty(OUT_LEN, dtype=np.float32)
    inv_scale = np.float32(1.0) / SCALE
    for m in range(NCORES):
        # seg[p, s*256+r] -> block s*125+p; de-interleave then trim to 1024
        v = segs[m].reshape(P, NS, FS)[0:SL].transpose(1, 0, 2).reshape(-1)
        full[m * FPC * FS:(m + 1) * FPC * FS] = \
            v[:FPC * FS].astype(np.float32) * inv_scale

    # exact f32 recompute of the 6 edge blocks (corr < 2 there: the final
    # division amplifies fp16 error by up to ~1e5) and the global tail
    Fh = S_real[0:3] * a32[None, :] + S_imag[0:3] * b32[None, :]
    full[0:FS] = Fh[0, 0:FS]
    full[FS:2 * FS] = Fh[0, FS:2 * FS] + Fh[1, 0:FS]
    full[2 * FS:3 * FS] = Fh[0, 2 * FS:3 * FS] + Fh[1, FS:2 * FS] + Fh[2, 0:FS]
    Ft = S_real[NF - 3:] * a32[None, :] + S_imag[NF - 3:] * b32[None, :]
    full[NF * FS:NF * FS + FS] = Ft[0, 3 * FS:] + Ft[1, 2 * FS:3 * FS] + Ft[2, FS:2 * FS]
    full[NF * FS + FS:NF * FS + 2 * FS] = Ft[1, 3 * FS:] + Ft[2, 2 * FS:3 * FS]
    full[NF * FS + 2 * FS:] = Ft[2, 3 * FS:]

    if "corr" not in _cache:
        _cache["corr"] = np.maximum(_window_correction(), np.float32(1e-8))
    return full / _cache["corr"]


def kernel(S_real, S_imag):
    S_real = np.asarray(S_real, dtype=np.float32)
    S_imag = np.asarray(S_imag, dtype=np.float32)
    in_maps = make_in_maps(S_real, S_imag)
    nc = _get_nc()
    res = run_bass_kernel_spmd(nc, in_maps, list(range(NCORES)))
    segs = [res.results[m]["out_seg"] for m in range(NCORES)]
    return assemble_output(S_real, S_imag, segs)
